# revision 7
# baseline (speedup 1.0000x reference)
"""Complex-valued attention (magnitude-softmax with phase reconstruction) on 8 TRN2 cores.

Sharding: core c -> (batch b = c//2, query-half qh = c%2). No collectives:
each core computes a disjoint [512, 1024] slice of the output. The query
half is selected by permuting the token axis of x^T host-side so that each
core's queries are always columns 0:512 of its shard (SPMD: one graph).

Math (per head h, scale S = 1/8):
  w = x @ Wqkv^T                         (bf16 matmuls, fp32 accum)
  z = dots^T[k, q]                       computed TRANSPOSED via stacked
      re/im contraction so softmax ends up on the partition axis
  m2 = zre^2 + zim^2                     (custom DVE op reading PSUM direct)
  l = ln(m2+eps);  e1 = exp(0.5*l + ln(S)) = S*mag   (affine fused into ACT)
  e = exp(e1)  (softmax numerator; mags are bounded so no max-subtraction)
  ff = e * recip_approx(e1) = e/(S*mag)  (single fused custom DVE op)
  are = zre(PSUM) * ff ; aim = zimS * ff (DVE / GPSIMD split)
  oh^T = sum_k wcombo^T @ attnU^T ; normalized by S/sum_k e via
      gpsimd partition_broadcast of reciprocal_approx_fast(sum e) (no PSUM)
  out = oh @ Wout^T  (+bias, which is zero)

Slot-pipelined schedule: one slot per (head, key-tile-pair); dots at slot t,
elementwise chain spans t..t+1, oh/S matmuls lag 2 slots. Stage-1 is spread
across slots as 16-MM bursts sharing the zim PSUM ring. PSUM: zre ring2
(4 banks, PSUM-resident until `are`), zim ring1 (2), oh ring1 (1), S ring1 (1).
Engines per slot (~1024 cols): ACT zimS-evict+Ln+Exp+Exp ~4.1us; DVE
m2+ff+are+extras ~4.4us; GPSIMD aim+broadcast ~2.5us; PE ~3.9us.
"""

import os
import sys
import numpy as np
import ml_dtypes

sys.path.insert(0, "/opt/trn_rl_repo")

from contextlib import ExitStack

import concourse.bass as bass
import concourse.tile as tile
from concourse import bacc, mybir, dve_ops
from concourse.bass_utils import run_bass_kernel_spmd
from concourse.dve_spec import Spec, Src0, Src1, C0, C1, C2, Bin, sq, lower, _has_src1
from concourse.dve_spec import AluOp
from concourse.dve_uop import DveOpSpec


def _register_dve_op(name, spec):
    if name in dve_ops._SUB_OPCODE_FOR_NAME:
        return next(o for o in dve_ops.OPS if o.name == name)
    opcode = max(dve_ops._SUB_OPCODE_FOR_NAME.values()) + 1
    shas = {}
    for ver in ("v3", "v4"):
        uops = lower(spec, ver=ver)
        shas[ver] = DveOpSpec(name=name, opcode=opcode, uops=uops,
                              rd1_en=_has_src1(spec)).sha(ver)
    op = dve_ops.DveOp(name, spec, subdim=False, uops_sha=shas)
    dve_ops.OPS.append(op)
    dve_ops.CUSTOM_DVE_SPECS[name] = spec
    dve_ops._SUB_OPCODE_FOR_NAME[name] = opcode
    return op


def _register_sqmag():
    """Custom DVE op: out = Src0^2 + Src1^2 (reads both dots PSUM tiles)."""
    return _register_dve_op("TENSOR_SQMAG_ANT", Spec(
        body=sq(Src0) + sq(Src1),
        reference=lambda in0, in1, s0, s1, imm2:
            (in0.astype(np.float32) ** 2 + in1.astype(np.float32) ** 2),
    ))


def _register_ffused():
    """Custom DVE op: out = Src1 * recip_approx(Src0), one instruction.

    BITWISE_NOT exponent-flip seed + ONE Newton pass, distributed as
    (Src1*y0)*(C1 - Src0*y0) to fit the v3 stage budget. Constants
    jointly minimax-fit over the [-4.5,-4] interval that x*bitcast(~x)
    lands in: max rel err 1.7e-3 (vs 51 ULP for the 2-pass version)."""
    _not = Bin(AluOp.BITWISE_NOT, Src0, Src0)
    _y0 = _not * C0

    def _ref(in0, in1, s0, s1, imm2):
        not_x = (~np.asarray(in0, np.float32).view(np.int32)).view(np.float32)
        y0 = (not_x * np.float32(s0)).astype(np.float32)
        return ((np.asarray(in1, np.float32) * y0)
                * (np.float32(s1) - np.asarray(in0, np.float32) * y0))

    return _register_dve_op("TENSOR_MUL_RECIP_ANT", Spec(
        body=(Src1 * _y0) * (C1 - Src0 * _y0),
        reference=_ref,
    ))


FF_C0 = 0.23549864
FF_C1 = -2.00173258


SQMAG = _register_sqmag()
FFUSED = _register_ffused()


def _patch_act_tables():
    """Force exp/ln to resolve to the combined natural_log_exp_and_others set
    so the per-tile Ln->Exp alternation doesn't reload ACT tables (~2.7us each)."""
    import concourse.bacc as _bacc
    if getattr(_bacc, "_act_tables_patched", False):
        return
    orig = _bacc.get_activation_tables
    AFT = mybir.ActivationFunctionType

    def patched(arch):
        t = {k: set(v) for k, v in orig(arch).items()}
        for name, fns in t.items():
            if name != "natural_log_exp_and_others":
                fns.discard(AFT.Exp)
                fns.discard(AFT.Ln)
        return t

    _bacc.get_activation_tables = patched
    _bacc._act_tables_patched = True


_patch_act_tables()

B, N, D, H, DH = 4, 1024, 1024, 16, 64
E = H * DH          # 1024
NQ = 512            # queries per core
KT = 8              # key tiles of 128
DT = 8              # d (contraction) tiles of 128
ET = 8              # e tiles of 128 (2 heads each)
SCALE = DH ** -0.5  # 0.125
LN_S = float(np.log(SCALE))
EPS = 1e-20
T = H * (KT // 2)   # 64 slots

FP32 = mybir.dt.float32
BF16 = mybir.dt.bfloat16
AF = mybir.ActivationFunctionType
ALU = mybir.AluOpType
RC = dve_ops.RECIP_APPROX_FAST_CONSTS

_CACHE = {}


def build_graph():
    nc = bacc.Bacc("TRN2", target_bir_lowering=False, debug=False,
                   enable_asserts=False, num_devices=8)

    xTr_d = nc.declare_dram_parameter("xTr", [D, N], BF16, isOutput=False)
    xTi_d = nc.declare_dram_parameter("xTi", [D, N], BF16, isOutput=False)
    wqr_d = nc.declare_dram_parameter("wqTr", [D, E], BF16, isOutput=False)   # Wqkv_re.T
    wqi_d = nc.declare_dram_parameter("wqTi", [D, E], BF16, isOutput=False)   # Wqkv_im.T
    wqin_d = nc.declare_dram_parameter("wqTin", [D, E], BF16, isOutput=False)  # -Wqkv_im.T
    wo_d = nc.declare_dram_parameter("woS", [2 * E, D], BF16, isOutput=False)  # [Wout_re.T; Wout_im.T]
    our_d = nc.declare_dram_parameter("out_re", [NQ, D], FP32, isOutput=True)
    oui_d = nc.declare_dram_parameter("out_im", [NQ, D], FP32, isOutput=True)

    with tile.TileContext(nc) as tc, ExitStack() as ctx:
        const_pool = ctx.enter_context(tc.tile_pool(name="const", bufs=1))
        xpool = ctx.enter_context(tc.tile_pool(name="x", bufs=1))
        wqpool = ctx.enter_context(tc.tile_pool(name="wq", bufs=2))
        apool = ctx.enter_context(tc.tile_pool(name="A", bufs=1))
        bpool = ctx.enter_context(tc.tile_pool(name="B", bufs=2))
        wcpool = ctx.enter_context(tc.tile_pool(name="wc", bufs=2))
        ohpool = ctx.enter_context(tc.tile_pool(name="oh", bufs=1))
        wopool = ctx.enter_context(tc.tile_pool(name="wo", bufs=2))
        epool = ctx.enter_context(tc.tile_pool(name="elem", bufs=2))
        spool = ctx.enter_context(tc.tile_pool(name="sm", bufs=2))
        opool = ctx.enter_context(tc.tile_pool(name="ostage", bufs=1))
        pp = ctx.enter_context(tc.tile_pool(name="pd", bufs=1, space="PSUM"))
        psoh = ctx.enter_context(tc.tile_pool(name="psoh", bufs=1, space="PSUM"))
        psS = ctx.enter_context(tc.tile_pool(name="psS", bufs=1, space="PSUM"))

        # ---- constants ----
        ones_bf = const_pool.tile([128, 1], BF16, tag="ones")
        nc.gpsimd.memset(ones_bf[:], 1.0)
        eps_t = const_pool.tile([128, 1], FP32, tag="eps")
        nc.gpsimd.memset(eps_t[:], EPS)
        lnS_t = const_pool.tile([128, 1], FP32, tag="lnS")
        nc.gpsimd.memset(lnS_t[:], LN_S)

        # ---- resident loads ----
        xr = xpool.tile([128, DT, N], BF16, tag="xr")
        xi = xpool.tile([128, DT, N], BF16, tag="xi")
        nc.sync.dma_start(out=xr[:], in_=xTr_d.ap().rearrange("(t p) n -> p t n", p=128))
        nc.sync.dma_start(out=xi[:], in_=xTi_d.ap().rearrange("(t p) n -> p t n", p=128))

        A = []          # per-head [128 (wr 0:64 | wi 64:128), N] bf16; ring of 6
        wslabs = {}     # et -> (wr, wi, win)

        def s1_load(et):
            wr = wqpool.tile([128, DT, 128], BF16, tag="wsr")
            wi = wqpool.tile([128, DT, 128], BF16, tag="wsi")
            win = wqpool.tile([128, DT, 128], BF16, tag="wsin")
            esl = slice(et * 128, (et + 1) * 128)
            nc.sync.dma_start(out=wr[:], in_=wqr_d.ap()[:, esl].rearrange("(t p) n -> p t n", p=128))
            nc.sync.dma_start(out=wi[:], in_=wqi_d.ap()[:, esl].rearrange("(t p) n -> p t n", p=128))
            nc.sync.dma_start(out=win[:], in_=wqin_d.ap()[:, esl].rearrange("(t p) n -> p t n", p=128))
            wslabs[et] = (wr, wi, win)
            A.append(apool.tile([128, N], BF16, tag="A", name=f"A{2*et}", bufs=6))
            A.append(apool.tile([128, N], BF16, tag="A", name=f"A{2*et+1}", bufs=6))

        def s1_chunk(et, nch, part, ptag="zim"):
            """16 MMs producing a [128, 512] slice of w^T (re or im part) for
            e-tile et, token chunk nch; evicted into the A stacks."""
            wr, wi, win = wslabs[et]
            ps1 = pp.tile([128, 2, 512], FP32, tag=ptag,
                          bufs=(2 if ptag == "zre" else 1))
            nsl = slice(nch * 512, (nch + 1) * 512)
            for dt_ in range(DT):
                first, last = dt_ == 0, dt_ == DT - 1
                if part == 0:   # re: Wr^T xr + (-Wi^T) xi
                    nc.tensor.matmul(ps1[:, 0, :], wr[:, dt_, :], xr[:, dt_, nsl],
                                     start=first, stop=False)
                    nc.tensor.matmul(ps1[:, 0, :], win[:, dt_, :], xi[:, dt_, nsl],
                                     start=False, stop=last)
                else:           # im: Wi^T xr + Wr^T xi
                    nc.tensor.matmul(ps1[:, 0, :], wi[:, dt_, :], xr[:, dt_, nsl],
                                     start=first, stop=False)
                    nc.tensor.matmul(ps1[:, 0, :], wr[:, dt_, :], xi[:, dt_, nsl],
                                     start=False, stop=last)
            h0, h1 = 2 * et, 2 * et + 1
            rows = slice(0, 64) if part == 0 else slice(64, 128)
            if part == 0:       # re-chunks evicted on ACT
                nc.scalar.copy(A[h0][rows, nsl], ps1[0:64, 0, :])
                nc.scalar.copy(A[h1][rows, nsl], ps1[64:128, 0, :])
            else:               # im-chunks evicted on DVE
                nc.vector.tensor_copy(A[h0][rows, nsl], ps1[0:64, 0, :])
                nc.vector.tensor_copy(A[h1][rows, nsl], ps1[64:128, 0, :])

        # per-head prep: B stack + transposed wcombo tiles
        Bh = [None] * H
        wc1 = [None] * H
        wc2 = [None] * H

        def prep_head(h):
            Ah = A[h]
            Bt = bpool.tile([128, N], BF16, tag="B", name=f"B{h}", bufs=2)
            nc.vector.tensor_scalar_mul(Bt[0:64, :], Ah[64:128, :], -1.0)
            nc.vector.tensor_copy(Bt[64:128, :], Ah[0:64, :])
            Bh[h] = Bt
            w1 = wcpool.tile([128, KT, 128], BF16, tag="wc1", name=f"wc1_{h}", bufs=2)
            w2 = wcpool.tile([128, KT, 128], BF16, tag="wc2", name=f"wc2_{h}", bufs=2)
            nc.sync.dma_start(w1[:], Ah[:], transpose=True)
            nc.sync.dma_start(w2[:], Bt[:], transpose=True)
            wc1[h] = w1
            wc2[h] = w2

        # oh^T stacks for stage 4
        ohr = ohpool.tile([128, ET, NQ], BF16, tag="ohr")
        ohi = ohpool.tile([128, ET, NQ], BF16, tag="ohi")
        ohin = ohpool.tile([128, ET, NQ], BF16, tag="ohin")

        # ---- prologue: x/w loads, stage-1 for e-tiles 0-1, head 0 prep ----
        for et in (0, 1):
            s1_load(et)
            for nch in (0, 1):
                for part in (0, 1):
                    s1_chunk(et, nch, part, ptag=("zre" if part == 0 else "zim"))
        prep_head(0)

        # ---- slot-pipelined main loop ----
        st = {}   # slot -> dict of live tiles
        cur_psoh = [None]
        cur_psS = [None]

        def dots(t):
            h = t // 4
            j = t % 4
            Ah, Bt = A[h], Bh[h]
            zre = pp.tile([128, 2, 512], FP32, tag="zre", bufs=2)
            zim = pp.tile([128, 2, 512], FP32, tag="zim", bufs=1)
            for i in range(2):
                ksl = slice((2 * j + i) * 128, (2 * j + i + 1) * 128)
                nc.tensor.matmul(zre[:, i, :], Ah[:, ksl], Ah[:, 0:NQ],
                                 start=True, stop=True)
                nc.tensor.matmul(zim[:, i, :], Bt[:, ksl], Ah[:, 0:NQ],
                                 start=True, stop=True)
            st[t] = {"zre": zre, "zim": zim}

        def chain_a(t):
            s = st[t]
            zimS = epool.tile([128, 2, 512], BF16, tag="zimS", bufs=2)
            nc.scalar.copy(zimS[:], s["zim"][:])
            m2 = epool.tile([128, 2, 512], FP32, tag="m2", bufs=2)
            nc.vector._custom_dve(SQMAG, out=m2[:], in0=s["zre"][:], in1=zimS[:])
            s["zimS"], s["m2"] = zimS, m2

        def chain_b(t):
            """ff/are (DVE) + aim (GPSIMD) for slot t (runs one slot later)."""
            s = st[t]
            ff = epool.tile([128, 2, 512], BF16, tag="ff", bufs=3)
            nc.vector._custom_dve(FFUSED, out=ff[:], in0=s["e1"][:], in1=s["ee"][:],
                                  s0=FF_C0, s1=FF_C1, imm2=0.0)
            are = epool.tile([128, 2, 512], BF16, tag="are", bufs=4)
            nc.vector.tensor_mul(are[:], s["zre"][:], ff[:])
            aim = epool.tile([128, 2, 512], BF16, tag="aim", bufs=4)
            nc.gpsimd.tensor_mul(aim[:], s["zimS"][:], ff[:])
            s["ff"], s["are"], s["aim"] = ff, are, aim
            del s["zre"], s["zim"], s["zimS"]

        def chain_c(t):
            s = st[t]
            ll = epool.tile([128, 2, 512], FP32, tag="ll", bufs=2)
            nc.scalar.activation(ll[:], s["m2"][:], AF.Ln, bias=eps_t[:])
            e1 = epool.tile([128, 2, 512], FP32, tag="e1", bufs=3)
            nc.scalar.activation(e1[:], ll[:], AF.Exp, scale=0.5, bias=lnS_t[:])
            ee = epool.tile([128, 2, 512], BF16, tag="ee", bufs=4)
            nc.scalar.activation(ee[:], e1[:], AF.Exp)
            s["e1"], s["ee"] = e1, ee

        def oh_mms(t):
            h = t // 4
            j = t % 4
            s = st[t]
            if j == 0:
                cur_psoh[0] = psoh.tile([128, NQ], FP32, tag="oh", name=f"psoh{h}")
                cur_psS[0] = psS.tile([1, NQ], FP32, tag="S", name=f"psS{h}")
            ps_oh, ps_s = cur_psoh[0], cur_psS[0]
            for i in range(2):
                kt = 2 * j + i
                first, last = kt == 0, kt == KT - 1
                nc.tensor.matmul(ps_oh[:], wc1[h][:, kt, :], s["are"][:, i, :],
                                 start=first, stop=False)
                nc.tensor.matmul(ps_oh[:], wc2[h][:, kt, :], s["aim"][:, i, :],
                                 start=False, stop=last)
                nc.tensor.matmul(ps_s[:], ones_bf[:], s["ee"][:, i, :],
                                 start=first, stop=last)
            if j == 3:
                finalize(h)
            del st[t]

        def finalize(h):
            ps_oh, ps_s = cur_psoh[0], cur_psS[0]
            rs = spool.tile([1, NQ], FP32, tag="rs", bufs=2)
            nc.vector.reciprocal_approx_fast(out=rs[:], in_=ps_s[:])
            bb = spool.tile([128, NQ], FP32, tag="bb", bufs=2)
            nc.gpsimd.partition_broadcast(bb[:], rs[:])
            et2, half = h // 2, (h % 2) * 64
            hs = slice(half, half + 64)
            nc.vector.scalar_tensor_tensor(ohr[hs, et2, :], ps_oh[0:64, :], SCALE,
                                           bb[0:64, :], op0=ALU.mult, op1=ALU.mult)
            nc.vector.scalar_tensor_tensor(ohi[hs, et2, :], ps_oh[64:128, :], SCALE,
                                           bb[64:128, :], op0=ALU.mult, op1=ALU.mult)
            nc.vector.scalar_tensor_tensor(ohin[hs, et2, :], ps_oh[64:128, :], -SCALE,
                                           bb[64:128, :], op0=ALU.mult, op1=ALU.mult)

        # slot loop
        for t in range(T + 2):
            if t < T:
                dots(t)
                chain_a(t)
            if t >= 1 and t - 1 < T:
                chain_b(t - 1)
            if t < T:
                chain_c(t)
            if t >= 2:
                oh_mms(t - 2)
            if t < T and t % 4 == 0 and (t // 4) + 1 < H:
                prep_head((t // 4) + 1)
            # stage-1 chunks for e-tiles 2-7, one 16-MM burst every 2 slots
            if t < 48 and t % 2 == 0:
                c = t // 2
                et, sub = 2 + c // 4, c % 4
                if sub == 0:
                    s1_load(et)
                s1_chunk(et, sub // 2, sub % 2)
            # wos prefetch for stage 4
            if t == 56 or t == 58:
                dc = (t - 56) // 2
                wos = wopool.tile([128, 16, 512], BF16, tag="wos", name=f"wos{dc}", bufs=2)
                dsl = slice(dc * 512, (dc + 1) * 512)
                nc.sync.dma_start(out=wos[:], in_=wo_d.ap()[:, dsl].rearrange("(t p) n -> p t n", p=128))
                wslabs[f"wos{dc}"] = wos

        # ---- stage 4: out = oh @ Wout^T ----
        for dc in range(2):
            dsl = slice(dc * 512, (dc + 1) * 512)
            wos = wslabs[f"wos{dc}"]
            for qt in range(4):
                qsl = slice(qt * 128, (qt + 1) * 128)
                po = pp.tile([128, 2, 512], FP32, tag="zre", bufs=2)
                for et in range(ET):
                    first = et == 0
                    nc.tensor.matmul(po[:, 0, :], ohr[:, et, qsl], wos[:, et, :],
                                     start=first, stop=False)
                    nc.tensor.matmul(po[:, 0, :], ohin[:, et, qsl], wos[:, ET + et, :],
                                     start=False, stop=(et == ET - 1))
                    nc.tensor.matmul(po[:, 1, :], ohi[:, et, qsl], wos[:, et, :],
                                     start=first, stop=False)
                    nc.tensor.matmul(po[:, 1, :], ohr[:, et, qsl], wos[:, ET + et, :],
                                     start=False, stop=(et == ET - 1))
                o_st = opool.tile([128, 2, 512], FP32, tag="ost", bufs=2)
                nc.scalar.copy(o_st[:], po[:])
                nc.sync.dma_start(out=our_d.ap()[qsl, dsl], in_=o_st[:, 0, :])
                nc.sync.dma_start(out=oui_d.ap()[qsl, dsl], in_=o_st[:, 1, :])

    nc.compile()
    return nc


def _to_bf16(a):
    return np.asarray(a, dtype=np.float32).astype(ml_dtypes.bfloat16)


def make_in_maps(x_re, x_im, wqkv_re, wqkv_im, wout_re, wout_im, bout_re, bout_im):
    x_re = np.asarray(x_re, np.float32)
    x_im = np.asarray(x_im, np.float32)
    wq_r = _to_bf16(np.asarray(wqkv_re, np.float32).T)
    wq_i = _to_bf16(np.asarray(wqkv_im, np.float32).T)
    wq_in = _to_bf16(-np.asarray(wqkv_im, np.float32).T)
    wo_s = _to_bf16(np.concatenate([np.asarray(wout_re, np.float32).T,
                                    np.asarray(wout_im, np.float32).T], axis=0))

    in_maps = []
    for c in range(8):
        b, qh = c // 2, c % 2
        xtr = x_re[b].T
        xti = x_im[b].T
        if qh == 1:
            xtr = np.concatenate([xtr[:, NQ:], xtr[:, :NQ]], axis=1)
            xti = np.concatenate([xti[:, NQ:], xti[:, :NQ]], axis=1)
        in_maps.append({
            "xTr": _to_bf16(np.ascontiguousarray(xtr)),
            "xTi": _to_bf16(np.ascontiguousarray(xti)),
            "wqTr": wq_r, "wqTi": wq_i, "wqTin": wq_in, "woS": wo_s,
        })
    return in_maps


def assemble_output(res, bout_re, bout_im):
    out = np.zeros((B, N, D), np.complex64)
    for c in range(8):
        b, qh = c // 2, c % 2
        rows = slice(0, NQ) if qh == 0 else slice(NQ, N)
        out[b, rows, :] = res[c]["out_re"] + 1j * res[c]["out_im"]

    # bout is zero in this problem; add anyway for faithfulness
    out += (np.asarray(bout_re, np.float32) + 1j * np.asarray(bout_im, np.float32))[None, None, :]
    return out


def kernel(**inputs):
    if "nc" not in _CACHE:
        _CACHE["nc"] = build_graph()
    nc = _CACHE["nc"]
    in_maps = make_in_maps(**inputs)
    res = run_bass_kernel_spmd(nc, in_maps, core_ids=list(range(8))).results
    return assemble_output(res, inputs["bout_re"], inputs["bout_im"])


# revision 10
# speedup vs baseline: 1.5636x; 1.5636x over previous
"""Complex-valued attention (magnitude-softmax with phase reconstruction) on 8 TRN2 cores.

Sharding: core c -> (batch b = c//2, query-half qh = c%2). No collectives:
each core computes a disjoint [512, 1024] slice of the output. The query
half is selected by permuting the token axis of x^T host-side so that each
core's queries are always columns 0:512 of its shard (SPMD: one graph).

Math (per head h, scale S = 1/8):
  w = x @ Wqkv^T       stage-1 with host-interleaved per-head weight slabs
      [Wr_h|Wi_h], [-Wi_h|Wr_h] so each accumulation writes the A stack
      [wr(0:64)|wi(64:128), tokens] directly (one [128,512] eviction/chunk)
  z = dots^T[k, q]     re: A_k . A_q ; im: A_k . Bq where Bq=[wi;-wr]_q
                       (shared stationary operand, moving-side B)
  m2 = zre^2 + zimS^2  custom DVE op, zre read from PSUM direct
  l = ln(m2+eps); e1 = exp(0.5*l + ln(S)) = S*mag; e = exp(e1)   (ACT)
  ff = e * recip_approx(e1)    single fused custom DVE op (1 Newton pass)
  are = zre(PSUM)*ff (DVE) ; aim = zimS*ff (GPSIMD)
  oh^T = sum_k wc^T @ attnU^T ; wc1 = A^T via DMA transpose (1/head),
      wc2 derived on-chip (column swap+negate of wc1); normalization via
      ones-matmul broadcast of reciprocal(sum e) (bb), ohraw evicted bf16
  out = oh @ Wout^T  (+bias, which is zero)

Slot pipeline: slot t=(head,kp). dots(t) -> chain spans t..t+1 -> oh lags 2.
Stage-1 chunks (16 MMs, one per 2 slots) share the zim PSUM ring. PSUM: zre
ring2 (4 banks, live until `are`), zim ring1 (2), oh ring1 (1), S/bb ring1 (1).
"""

import os
import sys
import numpy as np
import ml_dtypes

sys.path.insert(0, "/opt/trn_rl_repo")

from contextlib import ExitStack

import concourse.bass as bass
import concourse.tile as tile
from concourse import bacc, mybir, dve_ops
from concourse.bass_utils import run_bass_kernel_spmd
from concourse.dve_spec import Spec, Src0, Src1, C0, C1, C2, Bin, sq, lower, _has_src1
from concourse.dve_spec import AluOp
from concourse.dve_uop import DveOpSpec


def _register_dve_op(name, spec):
    if name in dve_ops._SUB_OPCODE_FOR_NAME:
        return next(o for o in dve_ops.OPS if o.name == name)
    opcode = max(dve_ops._SUB_OPCODE_FOR_NAME.values()) + 1
    shas = {}
    for ver in ("v3", "v4"):
        uops = lower(spec, ver=ver)
        shas[ver] = DveOpSpec(name=name, opcode=opcode, uops=uops,
                              rd1_en=_has_src1(spec)).sha(ver)
    op = dve_ops.DveOp(name, spec, subdim=False, uops_sha=shas)
    dve_ops.OPS.append(op)
    dve_ops.CUSTOM_DVE_SPECS[name] = spec
    dve_ops._SUB_OPCODE_FOR_NAME[name] = opcode
    return op


def _register_sqmag():
    """Custom DVE op: out = Src0^2 + Src1^2 (Src0 may be PSUM)."""
    return _register_dve_op("TENSOR_SQMAG_ANT", Spec(
        body=sq(Src0) + sq(Src1),
        reference=lambda in0, in1, s0, s1, imm2:
            (in0.astype(np.float32) ** 2 + in1.astype(np.float32) ** 2),
    ))


def _register_ffused():
    """Custom DVE op: out = Src1 * recip_approx(Src0), one instruction.

    BITWISE_NOT exponent-flip seed + ONE Newton pass, distributed as
    (Src1*y0)*(C1 - Src0*y0) to fit the v3 stage budget. Constants
    jointly minimax-fit over the [-4.5,-4] interval that x*bitcast(~x)
    lands in: max rel err 1.7e-3."""
    _not = Bin(AluOp.BITWISE_NOT, Src0, Src0)
    _y0 = _not * C0

    def _ref(in0, in1, s0, s1, imm2):
        not_x = (~np.asarray(in0, np.float32).view(np.int32)).view(np.float32)
        y0 = (not_x * np.float32(s0)).astype(np.float32)
        return ((np.asarray(in1, np.float32) * y0)
                * (np.float32(s1) - np.asarray(in0, np.float32) * y0))

    return _register_dve_op("TENSOR_MUL_RECIP_ANT", Spec(
        body=(Src1 * _y0) * (C1 - Src0 * _y0),
        reference=_ref,
    ))


FF_C0 = 0.23549864
FF_C1 = -2.00173258

SQMAG = _register_sqmag()
FFUSED = _register_ffused()


def _patch_act_tables():
    """Force exp/ln to resolve to the combined natural_log_exp_and_others set
    so the per-tile Ln->Exp alternation doesn't reload ACT tables (~2.7us each)."""
    import concourse.bacc as _bacc
    if getattr(_bacc, "_act_tables_patched", False):
        return
    orig = _bacc.get_activation_tables
    AFT = mybir.ActivationFunctionType

    def patched(arch):
        t = {k: set(v) for k, v in orig(arch).items()}
        for name, fns in t.items():
            if name != "natural_log_exp_and_others":
                fns.discard(AFT.Exp)
                fns.discard(AFT.Ln)
        return t

    _bacc.get_activation_tables = patched
    _bacc._act_tables_patched = True


_patch_act_tables()

B, N, D, H, DH = 4, 1024, 1024, 16, 64
E = H * DH          # 1024
NQ = 512            # queries per core
KT = 8              # key tiles of 128
DT = 8              # d (contraction) tiles of 128
ET = 8              # e tiles of 128 (2 heads each)
SCALE = DH ** -0.5  # 0.125
LN_S = float(np.log(SCALE))
EPS = 1e-20
T = H * (KT // 2)   # 64 slots

FP32 = mybir.dt.float32
BF16 = mybir.dt.bfloat16
AF = mybir.ActivationFunctionType
ALU = mybir.AluOpType

_CACHE = {}


def build_graph():
    nc = bacc.Bacc("TRN2", target_bir_lowering=False, debug=False,
                   enable_asserts=False, num_devices=8)

    xTr_d = nc.declare_dram_parameter("xTr", [D, N], BF16, isOutput=False)
    xTi_d = nc.declare_dram_parameter("xTi", [D, N], BF16, isOutput=False)
    # per-head interleaved stage-1 weights: wq1 col-block h = [Wr_h | Wi_h],
    # wq2 col-block h = [-Wi_h | Wr_h]  (each [D, 128])
    wq1_d = nc.declare_dram_parameter("wq1", [D, 2 * E], BF16, isOutput=False)
    wq2_d = nc.declare_dram_parameter("wq2", [D, 2 * E], BF16, isOutput=False)
    wo_d = nc.declare_dram_parameter("woS", [2 * E, D], BF16, isOutput=False)  # [Wout_re.T; Wout_im.T]
    our_d = nc.declare_dram_parameter("out_re", [NQ, D], FP32, isOutput=True)
    oui_d = nc.declare_dram_parameter("out_im", [NQ, D], FP32, isOutput=True)

    with tile.TileContext(nc) as tc, ExitStack() as ctx:
        const_pool = ctx.enter_context(tc.tile_pool(name="const", bufs=1))
        xpool = ctx.enter_context(tc.tile_pool(name="x", bufs=1))
        wqpool = ctx.enter_context(tc.tile_pool(name="wq", bufs=2))
        apool = ctx.enter_context(tc.tile_pool(name="A", bufs=1))
        bpool = ctx.enter_context(tc.tile_pool(name="B", bufs=2))
        wcpool = ctx.enter_context(tc.tile_pool(name="wc", bufs=1))
        ohpool = ctx.enter_context(tc.tile_pool(name="oh", bufs=1))
        wopool = ctx.enter_context(tc.tile_pool(name="wo", bufs=2))
        epool = ctx.enter_context(tc.tile_pool(name="elem", bufs=2))
        spool = ctx.enter_context(tc.tile_pool(name="sm", bufs=2))
        opool = ctx.enter_context(tc.tile_pool(name="ostage", bufs=2))
        pp = ctx.enter_context(tc.tile_pool(name="pd", bufs=1, space="PSUM"))
        psoh = ctx.enter_context(tc.tile_pool(name="psoh", bufs=1, space="PSUM"))
        psS = ctx.enter_context(tc.tile_pool(name="psS", bufs=1, space="PSUM"))

        # ---- constants ----
        ones_bf = const_pool.tile([128, 1], BF16, tag="ones")
        nc.gpsimd.memset(ones_bf[:], 1.0)
        eps_t = const_pool.tile([128, 1], FP32, tag="eps")
        nc.gpsimd.memset(eps_t[:], EPS)
        lnS_t = const_pool.tile([128, 1], FP32, tag="lnS")
        nc.gpsimd.memset(lnS_t[:], LN_S)
        ones8 = const_pool.tile([1, 128], FP32, tag="ones8")
        nc.gpsimd.memset(ones8[:], SCALE)

        # ---- resident loads ----
        xr = xpool.tile([128, DT, N], BF16, tag="xr")
        xi = xpool.tile([128, DT, N], BF16, tag="xi")
        nc.sync.dma_start(out=xr[:], in_=xTr_d.ap().rearrange("(t p) n -> p t n", p=128))
        nc.sync.dma_start(out=xi[:], in_=xTi_d.ap().rearrange("(t p) n -> p t n", p=128))

        A = [None] * H       # per-head [128 (wr|wi), N] bf16; ring6
        slabs = {}           # h -> (w1, w2)
        Bq = [None] * H      # [128 (wi|-wr), NQ] moving operand for zim dots
        wc1 = [None] * H     # [128 k, KT, 128 dh-stack] transposed A
        wc2 = [None] * H

        def s1_load(h):
            w1 = wqpool.tile([128, DT, 128], BF16, tag="ws1", name=f"w1_{h}")
            w2 = wqpool.tile([128, DT, 128], BF16, tag="ws2", name=f"w2_{h}")
            hsl = slice(h * 128, (h + 1) * 128)
            nc.sync.dma_start(out=w1[:], in_=wq1_d.ap()[:, hsl].rearrange("(t p) n -> p t n", p=128))
            nc.sync.dma_start(out=w2[:], in_=wq2_d.ap()[:, hsl].rearrange("(t p) n -> p t n", p=128))
            slabs[h] = (w1, w2)
            A[h] = apool.tile([128, N], BF16, tag="A", name=f"A{h}", bufs=6)

        def s1_chunk(h, nch, ptag="zim", evict_dve=False):
            """16 MMs producing A[h][:, nch*512:...] (one [128,512] eviction)."""
            w1, w2 = slabs[h]
            ps1 = pp.tile([128, 2, 512], FP32, tag=ptag, name=f"s1_{h}_{nch}",
                          bufs=(2 if ptag == "zre" else 1))
            nsl = slice(nch * 512, (nch + 1) * 512)
            for dt_ in range(DT):
                nc.tensor.matmul(ps1[:, 0, :], w1[:, dt_, :], xr[:, dt_, nsl],
                                 start=(dt_ == 0), stop=False)
                nc.tensor.matmul(ps1[:, 0, :], w2[:, dt_, :], xi[:, dt_, nsl],
                                 start=False, stop=(dt_ == DT - 1))
            if evict_dve:
                nc.vector.tensor_copy(A[h][:, nsl], ps1[0:128, 0, :])
            else:
                nc.scalar.copy(A[h][:, nsl], ps1[0:128, 0, :])

        def prep_wc1(h):
            w1t = wcpool.tile([128, KT, 128], BF16, tag="wc1", name=f"wc1_{h}", bufs=3)
            nc.sync.dma_start(w1t[:], A[h][:], transpose=True)
            wc1[h] = w1t

        def prep_head2(h):
            """wc2 derived from wc1 (col swap+negate); Bq moving operand."""
            w2t = wcpool.tile([128, KT, 128], BF16, tag="wc2", name=f"wc2_{h}", bufs=2)
            nc.vector.tensor_scalar_mul(w2t[:, :, 0:64], wc1[h][:, :, 64:128], -1.0)
            nc.vector.tensor_copy(w2t[:, :, 64:128], wc1[h][:, :, 0:64])
            wc2[h] = w2t
            bq = bpool.tile([128, NQ], BF16, tag="Bq", name=f"Bq{h}", bufs=2)
            nc.vector.tensor_copy(bq[0:64, :], A[h][64:128, 0:NQ])
            nc.vector.tensor_scalar_mul(bq[64:128, :], A[h][0:64, 0:NQ], -1.0)
            Bq[h] = bq

        # oh^T stacks for stage 4
        ohr = ohpool.tile([128, ET, NQ], BF16, tag="ohr")
        ohi = ohpool.tile([128, ET, NQ], BF16, tag="ohi")
        ohin = ohpool.tile([128, ET, NQ], BF16, tag="ohin")

        # static stage-1 schedule: head hh chunks at slots 4(hh-2)+1, +3
        s1_sched = {}
        for hh in range(3, H):
            s1_sched[4 * (hh - 3) + 1] = (hh, 0)
            s1_sched[4 * (hh - 3) + 3] = (hh, 1)

        # ---- prologue: x/w loads, heads 0-1 stage-1, head-0/1 wc prep ----
        for hh in (0, 1, 2, 3):
            s1_load(hh)
        for i, (hh, nch) in enumerate(((0, 0), (0, 1), (1, 0), (1, 1), (2, 0), (2, 1))):
            s1_chunk(hh, nch, ptag=("zre" if i % 2 else "zim"), evict_dve=(i % 2 == 1))
            if nch == 1:
                prep_wc1(hh)
        prep_head2(0)

        st = {}
        cur_psoh = [None]
        cur_psS = [None]

        def dots(t):
            h, j = t // 4, t % 4
            Ah, bq = A[h], Bq[h]
            zre = pp.tile([128, 2, 512], FP32, tag="zre", name=f"zre{t}", bufs=2)
            zim = pp.tile([128, 2, 512], FP32, tag="zim", name=f"zim{t}", bufs=1)
            for i in range(2):
                ksl = slice((2 * j + i) * 128, (2 * j + i + 1) * 128)
                nc.tensor.matmul(zre[:, i, :], Ah[:, ksl], Ah[:, 0:NQ],
                                 start=True, stop=True)
                nc.tensor.matmul(zim[:, i, :], Ah[:, ksl], bq[:],
                                 start=True, stop=True)
            st[t] = {"zre": zre, "zim": zim}

        def chain_a(t):
            s = st[t]
            zimS = epool.tile([128, 2, 512], BF16, tag="zimS", bufs=4)
            nc.scalar.copy(zimS[:], s["zim"][:])
            m2 = epool.tile([128, 2, 512], FP32, tag="m2", bufs=2)
            nc.vector._custom_dve(SQMAG, out=m2[:], in0=s["zre"][:], in1=zimS[:])
            s["zimS"], s["m2"] = zimS, m2

        def chain_b(t):
            s = st[t]
            ff = epool.tile([128, 2, 512], BF16, tag="ff", bufs=4)
            nc.vector._custom_dve(FFUSED, out=ff[:], in0=s["e1"][:], in1=s["ee"][:],
                                  s0=FF_C0, s1=FF_C1, imm2=0.0)
            are = epool.tile([128, 2, 512], BF16, tag="are", bufs=5)
            nc.vector.tensor_mul(are[:], s["zre"][:], ff[:])
            aim = epool.tile([128, 2, 512], BF16, tag="aim", bufs=5)
            nc.gpsimd.tensor_mul(aim[:], s["zimS"][:], ff[:])
            s["are"], s["aim"] = are, aim

        def chain_c(t):
            s = st[t]
            ll = epool.tile([128, 2, 512], FP32, tag="ll", bufs=2)
            nc.scalar.activation(ll[:], s["m2"][:], AF.Ln, bias=eps_t[:])
            e1 = epool.tile([128, 2, 512], FP32, tag="e1", bufs=3)
            nc.scalar.activation(e1[:], ll[:], AF.Exp, scale=0.5, bias=lnS_t[:])
            ee = epool.tile([128, 2, 512], BF16, tag="ee", bufs=5)
            nc.scalar.activation(ee[:], e1[:], AF.Exp)
            s["e1"], s["ee"] = e1, ee

        def oh_mms(t):
            h, j = t // 4, t % 4
            s = st[t]
            if j == 0:
                cur_psoh[0] = psoh.tile([128, NQ], FP32, tag="oh", name=f"psoh{h}")
                cur_psS[0] = psS.tile([128, NQ], FP32, tag="S", name=f"psS{h}")
            ps_oh, ps_s = cur_psoh[0], cur_psS[0]
            for i in range(2):
                kt = 2 * j + i
                first, last = kt == 0, kt == KT - 1
                nc.tensor.matmul(ps_oh[:], wc1[h][:, kt, :], s["are"][:, i, :],
                                 start=first, stop=False)
                nc.tensor.matmul(ps_oh[:], wc2[h][:, kt, :], s["aim"][:, i, :],
                                 start=False, stop=last)
                nc.tensor.matmul(ps_s[0:1, :], ones_bf[:], s["ee"][:, i, :],
                                 start=first, stop=last)
            if j == 3:
                finalize(h)
            del st[t]

        def finalize(h):
            ps_oh, ps_s = cur_psoh[0], cur_psS[0]
            ohraw = spool.tile([128, NQ], BF16, tag="ohraw", bufs=2)
            nc.scalar.copy(ohraw[:], ps_oh[:])
            rs = spool.tile([1, NQ], FP32, tag="rs", bufs=2)
            nc.vector.reciprocal_approx_fast(out=rs[:], in_=ps_s[0:1, :])
            bbt = psS.tile([128, NQ], FP32, tag="S", name=f"bb{h}")
            nc.tensor.matmul(bbt[:], ones8[:], rs[:], start=True, stop=True)
            et2, half = h // 2, (h % 2) * 64
            hs = slice(half, half + 64)
            nc.vector.tensor_mul(ohr[hs, et2, :], ohraw[0:64, :], bbt[0:64, :])
            nc.vector.tensor_mul(ohi[hs, et2, :], ohraw[64:128, :], bbt[64:128, :])
            nc.vector.tensor_scalar_mul(ohin[hs, et2, :], ohi[hs, et2, :], -1.0)

        # ---- slot loop ----
        for t in range(T + 2):
            if t < T:
                dots(t)
                chain_a(t)
            if 1 <= t <= T:
                chain_b(t - 1)
            if t < T:
                chain_c(t)
            if t >= 2:
                oh_mms(t - 2)
            # stage-1 chunk + prep pipeline
            if t in s1_sched:
                hh, nch = s1_sched[t]
                if nch == 0 and hh + 1 < H:
                    s1_load(hh + 1)
                elif nch == 0:
                    pass
                s1_chunk(hh, nch, evict_dve=(t % 4 == 1))
                if nch == 1:
                    prep_wc1(hh)
            if t % 4 == 1 and (t // 4) + 1 < H:
                prep_head2((t // 4) + 1)
            # wos prefetch for stage 4 (split re/im slabs per dc)
            if t in (54, 56, 58, 60):
                k = (t - 54) // 2
                dc, ri = k // 2, k % 2
                wos = wopool.tile([128, 8, 512], BF16, tag=f"wos{ri}", name=f"wos{dc}{ri}", bufs=2)
                dsl = slice(dc * 512, (dc + 1) * 512)
                rsl = slice(ri * E, ri * E + E)
                nc.sync.dma_start(out=wos[:], in_=wo_d.ap()[rsl, dsl].rearrange("(t p) n -> p t n", p=128))
                slabs[f"wos{dc}{ri}"] = wos

        # ---- stage 4: out = oh @ Wout^T ----
        for dc in range(2):
            dsl = slice(dc * 512, (dc + 1) * 512)
            wosA = slabs[f"wos{dc}0"]
            wosB = slabs[f"wos{dc}1"]
            for qt in range(4):
                qsl = slice(qt * 128, (qt + 1) * 128)
                po = pp.tile([128, 2, 512], FP32, tag="zre", name=f"po{dc}{qt}", bufs=2)
                for et in range(ET):
                    first = et == 0
                    nc.tensor.matmul(po[:, 0, :], ohr[:, et, qsl], wosA[:, et, :],
                                     start=first, stop=False)
                    nc.tensor.matmul(po[:, 0, :], ohin[:, et, qsl], wosB[:, et, :],
                                     start=False, stop=(et == ET - 1))
                    nc.tensor.matmul(po[:, 1, :], ohi[:, et, qsl], wosA[:, et, :],
                                     start=first, stop=False)
                    nc.tensor.matmul(po[:, 1, :], ohr[:, et, qsl], wosB[:, et, :],
                                     start=False, stop=(et == ET - 1))
                o_re = opool.tile([128, 512], FP32, tag="ore", bufs=1)
                o_im = opool.tile([128, 512], FP32, tag="oim", bufs=1)
                nc.vector.tensor_copy(o_re[:], po[:, 0, :])
                nc.scalar.copy(o_im[:], po[:, 1, :])
                nc.sync.dma_start(out=our_d.ap()[qsl, dsl], in_=o_re[:])
                nc.sync.dma_start(out=oui_d.ap()[qsl, dsl], in_=o_im[:])

    nc.compile()
    return nc


def _to_bf16(a):
    return np.asarray(a, dtype=np.float32).astype(ml_dtypes.bfloat16)


def make_in_maps(x_re, x_im, wqkv_re, wqkv_im, wout_re, wout_im, bout_re, bout_im):
    x_re = np.asarray(x_re, np.float32)
    x_im = np.asarray(x_im, np.float32)
    wr = np.asarray(wqkv_re, np.float32).T   # [D, E]
    wi = np.asarray(wqkv_im, np.float32).T
    # per-head interleave: block h of wq1 = [Wr_h | Wi_h], wq2 = [-Wi_h | Wr_h]
    w1 = np.empty((D, 2 * E), np.float32)
    w2 = np.empty((D, 2 * E), np.float32)
    for h in range(H):
        c = slice(h * DH, (h + 1) * DH)
        b0, b1 = slice(h * 128, h * 128 + 64), slice(h * 128 + 64, (h + 1) * 128)
        w1[:, b0], w1[:, b1] = wr[:, c], wi[:, c]
        w2[:, b0], w2[:, b1] = -wi[:, c], wr[:, c]
    wq1 = _to_bf16(w1)
    wq2 = _to_bf16(w2)
    wo_s = _to_bf16(np.concatenate([np.asarray(wout_re, np.float32).T,
                                    np.asarray(wout_im, np.float32).T], axis=0))

    in_maps = []
    for c in range(8):
        b, qh = c // 2, c % 2
        xtr = x_re[b].T
        xti = x_im[b].T
        if qh == 1:
            xtr = np.concatenate([xtr[:, NQ:], xtr[:, :NQ]], axis=1)
            xti = np.concatenate([xti[:, NQ:], xti[:, :NQ]], axis=1)
        in_maps.append({
            "xTr": _to_bf16(np.ascontiguousarray(xtr)),
            "xTi": _to_bf16(np.ascontiguousarray(xti)),
            "wq1": wq1, "wq2": wq2, "woS": wo_s,
        })
    return in_maps


def assemble_output(res, bout_re, bout_im):
    out = np.zeros((B, N, D), np.complex64)
    for c in range(8):
        b, qh = c // 2, c % 2
        rows = slice(0, NQ) if qh == 0 else slice(NQ, N)
        out[b, rows, :] = res[c]["out_re"] + 1j * res[c]["out_im"]

    # bout is zero in this problem; add anyway for faithfulness
    out += (np.asarray(bout_re, np.float32) + 1j * np.asarray(bout_im, np.float32))[None, None, :]
    return out


def kernel(**inputs):
    if "nc" not in _CACHE:
        _CACHE["nc"] = build_graph()
    nc = _CACHE["nc"]
    in_maps = make_in_maps(**inputs)
    res = run_bass_kernel_spmd(nc, in_maps, core_ids=list(range(8))).results
    return assemble_output(res, inputs["bout_re"], inputs["bout_im"])
